# revision 5
# baseline (speedup 1.0000x reference)
"""Trainium2 Bass kernel for SAGAN-style spatial self-attention.

Reference computation (per batch b):
    xf = x[b].reshape(C, N)                    # C=256, N=64*64=4096
    f  = w1 @ xf                               # [32, N]   (query^T)
    g  = w2 @ xf                               # [32, N]   (key)
    V  = (w3 @ xf)^T                           # [N, C]    (value)
    S  = f^T @ g                               # [N, N]
    O  = softmax(S, axis=-1) @ V               # [N, C]
    out[b] = O^T.reshape(C, H, W) + x[b]

Sharding: 8 cores = 4 batches x 2 query-halves. Each core holds its batch's
full xf (for keys/values) and computes attention for 2048 query positions.
No cross-core communication.

Per-core device algorithm (n = this core's 2048 query cols, m = all 4096 keys):
  - projections f [32,2048], g [32,4096] in fp32r; V [4096,257] in bf16
    (column 256 of V is ones -> PV matmul emits softmax denominator for free)
  - S^T chunks: matmul(lhsT=g_mtile [32,128], rhs=f_chunk [32,512]) -> PSUM
  - P^T = exp(S^T) -> SBUF bf16 (no max subtraction: |S| <~ 30, exp fits fp32)
  - O chunk: matmul(lhsT=P^T [128m,128n], rhs=V [128m,257]) accumulated over
    32 m-tiles -> [128n, 257]; r = 1/col256; O *= r
  - PE-transpose O to [C, n] layout, add residual xq, DMA out.
"""

import sys

sys.path.insert(0, "/opt/trn_rl_repo")

from contextlib import ExitStack

import numpy as np

import concourse.bass as bass
import concourse.tile as tile
from concourse import bacc, mybir
from concourse.bass import ts, ds
from concourse.bass_utils import run_bass_kernel_spmd
from concourse.masks import make_identity

F32 = mybir.dt.float32
F32R = mybir.dt.float32r
BF16 = mybir.dt.bfloat16

B, C, H, W = 4, 256, 64, 64
N = H * W          # 4096 keys per batch
NQ = N // 2        # 2048 queries per core
CK = 32            # query/key head dim
MT = N // 128      # 32 m-tiles
NCHUNK = NQ // 512  # 4 n-chunks of 512 query cols
EXP = mybir.ActivationFunctionType.Exp


def build_nc():
    nc = bacc.Bacc("TRN2", target_bir_lowering=False, debug=False, num_devices=8)
    xkv_d = nc.dram_tensor("xkv", [C, N], F32, kind="ExternalInput")
    xq_d = nc.dram_tensor("xq", [C, NQ], F32, kind="ExternalInput")
    w1t_d = nc.dram_tensor("w1t", [C, CK], F32, kind="ExternalInput")
    w2t_d = nc.dram_tensor("w2t", [C, CK], F32, kind="ExternalInput")
    w3t_d = nc.dram_tensor("w3t", [C, C], F32, kind="ExternalInput")
    out_d = nc.dram_tensor("out", [C, NQ], F32, kind="ExternalOutput")

    with tile.TileContext(nc) as tc, ExitStack() as ctx:
        _body(ctx, tc, xkv_d.ap(), xq_d.ap(), w1t_d.ap(), w2t_d.ap(), w3t_d.ap(),
              out_d.ap())
    nc.compile()
    return nc


def _body(ctx, tc, xkv_d, xq_d, w1t_d, w2t_d, w3t_d, out_d):
    nc = tc.nc
    singles = ctx.enter_context(tc.tile_pool(name="singles", bufs=1))

    xq = singles.tile([128, 2, NQ], F32, tag="xq", name="xq")
    xkv_r = singles.tile([128, 2, N], F32R, tag="xkv_r", name="xkv_r")
    xq_r = singles.tile([128, 2, NQ], F32R, tag="xq_r", name="xq_r")
    w1t = singles.tile([128, 2, CK], F32R, tag="w1t", name="w1t")
    w2t = singles.tile([128, 2, CK], F32R, tag="w2t", name="w2t")
    w3t = singles.tile([128, 2, C], F32R, tag="w3t", name="w3t")
    g_sb = singles.tile([CK, N], F32R, tag="g_sb", name="g_sb")
    f_sb = singles.tile([CK, NQ], F32R, tag="f_sb", name="f_sb")
    V = singles.tile([128, MT, 260], BF16, tag="V", name="V")
    ident = singles.tile([128, 128], F32, tag="ident", name="ident")

    make_identity(nc, ident)
    nc.vector.memset(V[:, :, 256:257], 1.0)

    # fp32r matmul operands must come from an on-chip "rounding" producer, so
    # DMA raw fp32 into scratch and round-copy on GpSimd (idle engine).
    with tc.tile_pool(name="xraw", bufs=1) as xrp:
        xkv = xrp.tile([128, 2, N], F32, tag="xkv", name="xkv")
        wraw = xrp.tile([128, 2, 2 * CK + C], F32, tag="wraw", name="wraw")
        for k in range(2):
            nc.sync.dma_start(wraw[:, k, 0:CK], w1t_d[ts(k, 128), :])
            nc.sync.dma_start(wraw[:, k, CK:2 * CK], w2t_d[ts(k, 128), :])
            nc.sync.dma_start(wraw[:, k, 2 * CK:], w3t_d[ts(k, 128), :])
        nc.gpsimd.tensor_copy(w1t[:], wraw[:, :, 0:CK])
        nc.gpsimd.tensor_copy(w2t[:], wraw[:, :, CK:2 * CK])
        nc.gpsimd.tensor_copy(w3t[:], wraw[:, :, 2 * CK:])
        for ch in range(N // 512):
            for k in range(2):
                nc.sync.dma_start(xkv[:, k, ts(ch, 512)],
                                  xkv_d[ts(k, 128), ts(ch, 512)])
            nc.gpsimd.tensor_copy(xkv_r[:, :, ts(ch, 512)], xkv[:, :, ts(ch, 512)])
        for ch in range(NQ // 512):
            for k in range(2):
                nc.sync.dma_start(xq[:, k, ts(ch, 512)], xq_d[ts(k, 128), ts(ch, 512)])
            nc.gpsimd.tensor_copy(xq_r[:, :, ts(ch, 512)], xq[:, :, ts(ch, 512)])

        # ---- projections ----
        with tc.tile_pool(name="vproj_ps", bufs=2, space="PSUM") as pvp, \
             tc.tile_pool(name="proj_ps", bufs=2, space="PSUM") as pp:
            # V first: its PSUM->SBUF copies run on ACT (idle until exp work),
            # overlapping with the g/f projections below on PE.
            for mt in range(MT):
                vp = pvp.tile([128, 256], F32, tag="vp", name="vp")
                for k in range(2):
                    nc.tensor.matmul(vp[:], xkv_r[:, k, ts(mt, 128)], w3t[:, k, :],
                                     start=(k == 0), stop=(k == 1))
                nc.scalar.activation(V[:, mt, 0:256], vp[:],
                                     mybir.ActivationFunctionType.Copy)
            for ch in range(N // 512):
                gp = pp.tile([CK, 512], F32, tag="gp", name="gp")
                for k in range(2):
                    nc.tensor.matmul(gp[:], w2t[:, k, :], xkv_r[:, k, ts(ch, 512)],
                                     start=(k == 0), stop=(k == 1))
                nc.vector.tensor_copy(g_sb[:, ts(ch, 512)], gp[:])
            for ch in range(NQ // 512):
                fp = pp.tile([CK, 512], F32, tag="fp", name="fp")
                for k in range(2):
                    nc.tensor.matmul(fp[:], w1t[:, k, :], xq_r[:, k, ts(ch, 512)],
                                     start=(k == 0), stop=(k == 1))
                nc.vector.tensor_copy(f_sb[:, ts(ch, 512)], fp[:])

    # ---- attention, software-pipelined by one chunk ----
    # chunk c: produce S^T/P^T for n-cols [c*512, (c+1)*512); consume P^T of
    # chunk c-1 with the PV matmul, interleaved at group granularity so PE
    # never waits on the exp (ACT) stream.
    stp = ctx.enter_context(tc.tile_pool(name="st_ps", bufs=2, space="PSUM"))
    op = ctx.enter_context(tc.tile_pool(name="o_ps", bufs=2, space="PSUM"))
    tp = ctx.enter_context(tc.tile_pool(name="t_ps", bufs=2, space="PSUM"))
    ptp = ctx.enter_context(tc.tile_pool(name="pt", bufs=2))
    osbp = ctx.enter_context(tc.tile_pool(name="osb", bufs=2))
    rp = ctx.enter_context(tc.tile_pool(name="r", bufs=2))
    stgp = ctx.enter_context(tc.tile_pool(name="stage", bufs=2))

    Pt = [None, None]
    stage = [None, None]
    posts = []

    def emit_post(item):
        cc, j, o_ps, stg = item
        J = cc * 4 + j
        r = rp.tile([128, 1], F32, tag="r", name="r")
        nc.vector.reciprocal(r[:], o_ps[:, 256:257])
        o_sb = osbp.tile([128, 256], F32, tag="osb", name="osb")
        nc.vector.tensor_scalar_mul(o_sb[:], o_ps[:, 0:256], r[:])
        for h in range(2):
            t_ps = tp.tile([128, 128], F32, tag="t", name="t")
            nc.tensor.transpose(t_ps[:], o_sb[:, ts(h, 128)], ident[:])
            nc.vector.tensor_add(stg[:, h, ts(j, 128)], t_ps[:],
                                 xq[:, h, ds(J * 128, 128)])
        if j == 3:
            for k in range(2):
                nc.sync.dma_start(out_d[ts(k, 128), ts(cc, 512)], stg[:, k, :])

    for c in range(NCHUNK + 1):
        if c < NCHUNK:
            Pt[c % 2] = ptp.tile([128, MT, 512], BF16, tag="pt", name="pt")
        if c >= 1:
            stage[(c - 1) % 2] = stgp.tile([128, 2, 512], F32, tag="stage", name="stage")
        o_cur = None
        for gidx in range(16):
            if c < NCHUNK:
                st = stp.tile([128, 2, 512], F32, tag="st", name="st")
                for t in range(2):
                    mt = 2 * gidx + t
                    nc.tensor.matmul(st[:, t, :], g_sb[:, ts(mt, 128)],
                                     f_sb[:, ts(c, 512)], start=True, stop=True)
                nc.scalar.activation(Pt[c % 2][:, 2 * gidx:2 * gidx + 2, :], st[:], EXP)
            if c >= 1:
                j, seg = gidx // 4, gidx % 4
                if seg == 0:
                    o_cur = op.tile([128, 257], F32, tag="o", name="o")
                for mm in range(8):
                    mt = seg * 8 + mm
                    nc.tensor.matmul(o_cur[:], Pt[(c - 1) % 2][:, mt, ts(j, 128)],
                                     V[:, mt, 0:257],
                                     start=(mt == 0), stop=(mt == MT - 1),
                                     skip_group_check=True)
                if seg == 3:
                    posts.append((c - 1, j, o_cur, stage[(c - 1) % 2]))
            # delay each n-tile's post-processing by a few PE groups so the
            # DVE normalize never stalls the PE stream
            while len(posts) > 1:
                emit_post(posts.pop(0))
    while posts:
        emit_post(posts.pop(0))


_NC_CACHE = None


def _get_nc():
    global _NC_CACHE
    if _NC_CACHE is None:
        _NC_CACHE = build_nc()
    return _NC_CACHE


def make_in_maps(x, w1, w2, w3):
    x = np.ascontiguousarray(x, dtype=np.float32).reshape(B, C, N)
    w1t = np.ascontiguousarray(w1.T, dtype=np.float32)
    w2t = np.ascontiguousarray(w2.T, dtype=np.float32)
    w3t = np.ascontiguousarray(w3.T, dtype=np.float32)
    in_maps = []
    for core in range(8):
        b, half = core // 2, core % 2
        in_maps.append({
            "xkv": x[b],
            "xq": np.ascontiguousarray(x[b][:, half * NQ:(half + 1) * NQ]),
            "w1t": w1t,
            "w2t": w2t,
            "w3t": w3t,
        })
    return in_maps


def assemble(results):
    out = np.empty((B, C, N), dtype=np.float32)
    for core in range(8):
        b, half = core // 2, core % 2
        out[b][:, half * NQ:(half + 1) * NQ] = results[core]["out"]
    return out.reshape(B, C, H, W)


def kernel(x, w1, w2, w3):
    nc = _get_nc()
    res = run_bass_kernel_spmd(nc, make_in_maps(x, w1, w2, w3),
                               core_ids=list(range(8)))
    return assemble(res.results)


# revision 6
# speedup vs baseline: 1.0480x; 1.0480x over previous
"""Trainium2 Bass kernel for SAGAN-style spatial self-attention.

Reference computation (per batch b):
    xf = x[b].reshape(C, N)                    # C=256, N=64*64=4096
    f  = w1 @ xf                               # [32, N]   (query^T)
    g  = w2 @ xf                               # [32, N]   (key)
    V  = (w3 @ xf)^T                           # [N, C]    (value)
    S  = f^T @ g                               # [N, N]
    O  = softmax(S, axis=-1) @ V               # [N, C]
    out[b] = O^T.reshape(C, H, W) + x[b]

Sharding: 8 cores = 4 batches x 2 query-halves. Each core holds its batch's
full xf (for keys/values) and computes attention for 2048 query positions.
No cross-core communication.

Per-core device algorithm (n = this core's 2048 query cols, m = all 4096 keys):
  - projections f [32,2048], g [32,4096] in fp32r; V [4096,257] in bf16
    (column 256 of V is ones -> PV matmul emits softmax denominator for free)
  - S^T chunks: matmul(lhsT=g_mtile [32,128], rhs=f_chunk [32,512]) -> PSUM
  - P^T = exp(S^T) -> SBUF bf16 (no max subtraction: |S| <~ 30, exp fits fp32)
  - O chunk: matmul(lhsT=P^T [128m,128n], rhs=V [128m,257]) accumulated over
    32 m-tiles -> [128n, 257]; r = 1/col256; O *= r
  - PE-transpose O to [C, n] layout, add residual xq, DMA out.
"""

import sys

sys.path.insert(0, "/opt/trn_rl_repo")

from contextlib import ExitStack

import numpy as np

import concourse.bass as bass
import concourse.tile as tile
from concourse import bacc, mybir
from concourse.bass import ts, ds
from concourse.bass_utils import run_bass_kernel_spmd
from concourse.masks import make_identity

F32 = mybir.dt.float32
F16 = mybir.dt.float16
BF16 = mybir.dt.bfloat16

B, C, H, W = 4, 256, 64, 64
N = H * W          # 4096 keys per batch
NQ = N // 2        # 2048 queries per core
CK = 32            # query/key head dim
MT = N // 128      # 32 m-tiles
NCHUNK = NQ // 512  # 4 n-chunks of 512 query cols
EXP = mybir.ActivationFunctionType.Exp


def build_nc():
    nc = bacc.Bacc("TRN2", target_bir_lowering=False, debug=False, num_devices=8)
    xkv_d = nc.dram_tensor("xkv", [C, N], F32, kind="ExternalInput")
    xq_d = nc.dram_tensor("xq", [C, NQ], F32, kind="ExternalInput")
    w1t_d = nc.dram_tensor("w1t", [C, CK], F32, kind="ExternalInput")
    w2t_d = nc.dram_tensor("w2t", [C, CK], F32, kind="ExternalInput")
    w3t_d = nc.dram_tensor("w3t", [C, C], F32, kind="ExternalInput")
    out_d = nc.dram_tensor("out", [C, NQ], F32, kind="ExternalOutput")

    with tile.TileContext(nc) as tc, ExitStack() as ctx:
        _body(ctx, tc, xkv_d.ap(), xq_d.ap(), w1t_d.ap(), w2t_d.ap(), w3t_d.ap(),
              out_d.ap())
    nc.compile()
    return nc


def _body(ctx, tc, xkv_d, xq_d, w1t_d, w2t_d, w3t_d, out_d):
    nc = tc.nc
    singles = ctx.enter_context(tc.tile_pool(name="singles", bufs=1))

    xq = singles.tile([128, 2, NQ], F32, tag="xq", name="xq")
    xkv_r = singles.tile([128, 2, N], F16, tag="xkv_r", name="xkv_r")
    xq_r = singles.tile([128, 2, NQ], F16, tag="xq_r", name="xq_r")
    w1t = singles.tile([128, 2, CK], F16, tag="w1t", name="w1t")
    w2t = singles.tile([128, 2, CK], F16, tag="w2t", name="w2t")
    w3t = singles.tile([128, 2, C], F16, tag="w3t", name="w3t")
    g_sb = singles.tile([CK, N], F16, tag="g_sb", name="g_sb")
    f_sb = singles.tile([CK, NQ], F16, tag="f_sb", name="f_sb")
    V = singles.tile([128, MT, 260], BF16, tag="V", name="V")
    ident = singles.tile([128, 128], F32, tag="ident", name="ident")

    make_identity(nc, ident)
    nc.vector.memset(V[:, :, 256:257], 1.0)

    # fp16 operands for all projection/score matmuls: fp32-mode (HIGH) matmuls
    # do not register as PE-busy for the HAM clock gate and keep the PE at
    # 1.2GHz; fp16 keeps full clock and has enough mantissa for the scores.
    # DMA raw fp32 into scratch and cast on GpSimd (idle engine).
    with tc.tile_pool(name="xraw", bufs=1) as xrp:
        xkv = xrp.tile([128, 2, N], F32, tag="xkv", name="xkv")
        wraw = xrp.tile([128, 2, 2 * CK + C], F32, tag="wraw", name="wraw")
        for k in range(2):
            nc.sync.dma_start(wraw[:, k, 0:CK], w1t_d[ts(k, 128), :])
            nc.sync.dma_start(wraw[:, k, CK:2 * CK], w2t_d[ts(k, 128), :])
            nc.sync.dma_start(wraw[:, k, 2 * CK:], w3t_d[ts(k, 128), :])
        nc.gpsimd.tensor_copy(w1t[:], wraw[:, :, 0:CK])
        nc.gpsimd.tensor_copy(w2t[:], wraw[:, :, CK:2 * CK])
        nc.gpsimd.tensor_copy(w3t[:], wraw[:, :, 2 * CK:])
        for ch in range(N // 512):
            for k in range(2):
                nc.sync.dma_start(xkv[:, k, ts(ch, 512)],
                                  xkv_d[ts(k, 128), ts(ch, 512)])
            nc.gpsimd.tensor_copy(xkv_r[:, :, ts(ch, 512)], xkv[:, :, ts(ch, 512)])
        for ch in range(NQ // 512):
            for k in range(2):
                nc.sync.dma_start(xq[:, k, ts(ch, 512)], xq_d[ts(k, 128), ts(ch, 512)])
            nc.gpsimd.tensor_copy(xq_r[:, :, ts(ch, 512)], xq[:, :, ts(ch, 512)])

        # ---- projections ----
        with tc.tile_pool(name="vproj_ps", bufs=2, space="PSUM") as pvp, \
             tc.tile_pool(name="proj_ps", bufs=2, space="PSUM") as pp:
            # V first: its PSUM->SBUF copies run on ACT (idle until exp work),
            # overlapping with the g/f projections below on PE.
            for mt in range(MT):
                vp = pvp.tile([128, 256], F32, tag="vp", name="vp")
                for k in range(2):
                    nc.tensor.matmul(vp[:], xkv_r[:, k, ts(mt, 128)], w3t[:, k, :],
                                     start=(k == 0), stop=(k == 1))
                nc.scalar.activation(V[:, mt, 0:256], vp[:],
                                     mybir.ActivationFunctionType.Copy)
            for ch in range(N // 512):
                gp = pp.tile([CK, 512], F32, tag="gp", name="gp")
                for k in range(2):
                    nc.tensor.matmul(gp[:], w2t[:, k, :], xkv_r[:, k, ts(ch, 512)],
                                     start=(k == 0), stop=(k == 1))
                nc.vector.tensor_copy(g_sb[:, ts(ch, 512)], gp[:])
            for ch in range(NQ // 512):
                fp = pp.tile([CK, 512], F32, tag="fp", name="fp")
                for k in range(2):
                    nc.tensor.matmul(fp[:], w1t[:, k, :], xq_r[:, k, ts(ch, 512)],
                                     start=(k == 0), stop=(k == 1))
                nc.vector.tensor_copy(f_sb[:, ts(ch, 512)], fp[:])

    # ---- attention, software-pipelined by one chunk ----
    # chunk c: produce S^T/P^T for n-cols [c*512, (c+1)*512); consume P^T of
    # chunk c-1 with the PV matmul, interleaved at group granularity so PE
    # never waits on the exp (ACT) stream.
    stp = ctx.enter_context(tc.tile_pool(name="st_ps", bufs=2, space="PSUM"))
    op = ctx.enter_context(tc.tile_pool(name="o_ps", bufs=2, space="PSUM"))
    tp = ctx.enter_context(tc.tile_pool(name="t_ps", bufs=2, space="PSUM"))
    ptp = ctx.enter_context(tc.tile_pool(name="pt", bufs=2))
    osbp = ctx.enter_context(tc.tile_pool(name="osb", bufs=2))
    rp = ctx.enter_context(tc.tile_pool(name="r", bufs=2))
    stgp = ctx.enter_context(tc.tile_pool(name="stage", bufs=2))

    Pt = [None, None]
    stage = [None, None]
    posts = []

    def emit_post(item):
        cc, j, o_ps, stg = item
        J = cc * 4 + j
        r = rp.tile([128, 1], F32, tag="r", name="r")
        nc.vector.reciprocal(r[:], o_ps[:, 256:257])
        o_sb = osbp.tile([128, 256], F32, tag="osb", name="osb")
        nc.vector.tensor_scalar_mul(o_sb[:], o_ps[:, 0:256], r[:])
        for h in range(2):
            t_ps = tp.tile([128, 128], F32, tag="t", name="t")
            nc.tensor.transpose(t_ps[:], o_sb[:, ts(h, 128)], ident[:])
            nc.vector.tensor_add(stg[:, h, ts(j, 128)], t_ps[:],
                                 xq[:, h, ds(J * 128, 128)])
        if j == 3:
            for k in range(2):
                nc.sync.dma_start(out_d[ts(k, 128), ts(cc, 512)], stg[:, k, :])

    for c in range(NCHUNK + 1):
        if c < NCHUNK:
            Pt[c % 2] = ptp.tile([128, MT, 512], BF16, tag="pt", name="pt")
        if c >= 1:
            stage[(c - 1) % 2] = stgp.tile([128, 2, 512], F32, tag="stage", name="stage")
        o_cur = None
        for gidx in range(16):
            if c < NCHUNK:
                st = stp.tile([128, 2, 512], F32, tag="st", name="st")
                for t in range(2):
                    mt = 2 * gidx + t
                    nc.tensor.matmul(st[:, t, :], g_sb[:, ts(mt, 128)],
                                     f_sb[:, ts(c, 512)], start=True, stop=True)
                nc.scalar.activation(Pt[c % 2][:, 2 * gidx:2 * gidx + 2, :], st[:], EXP)
            if c >= 1:
                j, seg = gidx // 4, gidx % 4
                if seg == 0:
                    o_cur = op.tile([128, 257], F32, tag="o", name="o")
                for mm in range(8):
                    mt = seg * 8 + mm
                    nc.tensor.matmul(o_cur[:], Pt[(c - 1) % 2][:, mt, ts(j, 128)],
                                     V[:, mt, 0:257],
                                     start=(mt == 0), stop=(mt == MT - 1),
                                     skip_group_check=True)
                if seg == 3:
                    posts.append((c - 1, j, o_cur, stage[(c - 1) % 2]))
            # delay each n-tile's post-processing by a few PE groups so the
            # DVE normalize never stalls the PE stream
            while len(posts) > 1:
                emit_post(posts.pop(0))
    while posts:
        emit_post(posts.pop(0))


_NC_CACHE = None


def _get_nc():
    global _NC_CACHE
    if _NC_CACHE is None:
        _NC_CACHE = build_nc()
    return _NC_CACHE


def make_in_maps(x, w1, w2, w3):
    x = np.ascontiguousarray(x, dtype=np.float32).reshape(B, C, N)
    w1t = np.ascontiguousarray(w1.T, dtype=np.float32)
    w2t = np.ascontiguousarray(w2.T, dtype=np.float32)
    w3t = np.ascontiguousarray(w3.T, dtype=np.float32)
    in_maps = []
    for core in range(8):
        b, half = core // 2, core % 2
        in_maps.append({
            "xkv": x[b],
            "xq": np.ascontiguousarray(x[b][:, half * NQ:(half + 1) * NQ]),
            "w1t": w1t,
            "w2t": w2t,
            "w3t": w3t,
        })
    return in_maps


def assemble(results):
    out = np.empty((B, C, N), dtype=np.float32)
    for core in range(8):
        b, half = core // 2, core % 2
        out[b][:, half * NQ:(half + 1) * NQ] = results[core]["out"]
    return out.reshape(B, C, H, W)


def kernel(x, w1, w2, w3):
    nc = _get_nc()
    res = run_bass_kernel_spmd(nc, make_in_maps(x, w1, w2, w3),
                               core_ids=list(range(8)))
    return assemble(res.results)


# revision 7
# speedup vs baseline: 1.4073x; 1.3429x over previous
"""Trainium2 Bass kernel for SAGAN-style spatial self-attention.

Reference computation (per batch b):
    xf = x[b].reshape(C, N)                    # C=256, N=64*64=4096
    f  = w1 @ xf                               # [32, N]   (query^T)
    g  = w2 @ xf                               # [32, N]   (key)
    V  = (w3 @ xf)^T                           # [N, C]    (value)
    S  = f^T @ g                               # [N, N]
    O  = softmax(S, axis=-1) @ V               # [N, C]
    out[b] = O^T.reshape(C, H, W) + x[b]

Sharding: 8 cores = 4 batches x 2 query-halves. Each core holds its batch's
full xf (for keys/values) and computes attention for 2048 query positions.
No cross-core communication.

Per-core device algorithm (n = this core's 2048 query cols, m = all 4096 keys):
  - projections f [32,2048], g [32,4096] in fp16; V [4096,257] in bf16
    (column 256 of V is ones -> PV matmul emits softmax denominator for free)
  - S^T chunks: matmul(lhsT=g_mtile [32,128], rhs=f_chunk [32,512]) -> PSUM
  - P^T = exp(S^T) -> SBUF bf16 (no max subtraction: |S| <~ 45, exp fits fp32)
  - O chunk: matmul(lhsT=P^T [128m,128n], rhs=V [128m,257]) accumulated over
    32 m-tiles -> [128n, 257]; r = 1/col256; O *= r (bf16)
  - DMA-transpose O to [C, n] layout, add residual xq, DMA out.

fp16 (not fp32/fp32r) operands everywhere on the PE: fp32-mode matmuls do not
register as PE-busy for the HAM clock gate and the PE gets stuck at 1.2GHz.
fp16 keeps full clock and has enough mantissa (2^-11) for the pre-exp scores.
"""

import sys

sys.path.insert(0, "/opt/trn_rl_repo")

from contextlib import ExitStack

import numpy as np

import concourse.bass as bass
import concourse.tile as tile
from concourse import bacc, mybir
from concourse.bass import ts, ds
from concourse.bass_utils import run_bass_kernel_spmd

F32 = mybir.dt.float32
F16 = mybir.dt.float16
BF16 = mybir.dt.bfloat16

B, C, H, W = 4, 256, 64, 64
N = H * W          # 4096 keys per batch
NQ = N // 2        # 2048 queries per core
CK = 32            # query/key head dim
MT = N // 128      # 32 m-tiles
NCHUNK = NQ // 512  # 4 n-chunks of 512 query cols
EXP = mybir.ActivationFunctionType.Exp


def build_nc():
    nc = bacc.Bacc("TRN2", target_bir_lowering=False, debug=False, num_devices=8)
    xkv_d = nc.dram_tensor("xkv", [C, N], F32, kind="ExternalInput")
    xq_d = nc.dram_tensor("xq", [C, NQ], F32, kind="ExternalInput")
    w1t_d = nc.dram_tensor("w1t", [C, CK], F32, kind="ExternalInput")
    w2t_d = nc.dram_tensor("w2t", [C, CK], F32, kind="ExternalInput")
    w3t_d = nc.dram_tensor("w3t", [C, C], F32, kind="ExternalInput")
    out_d = nc.dram_tensor("out", [C, NQ], F32, kind="ExternalOutput")

    with tile.TileContext(nc) as tc, ExitStack() as ctx:
        _body(ctx, tc, xkv_d.ap(), xq_d.ap(), w1t_d.ap(), w2t_d.ap(), w3t_d.ap(),
              out_d.ap())
    nc.compile()
    return nc


def _body(ctx, tc, xkv_d, xq_d, w1t_d, w2t_d, w3t_d, out_d):
    nc = tc.nc
    singles = ctx.enter_context(tc.tile_pool(name="singles", bufs=1))

    xq = singles.tile([128, 2, NQ], F32, tag="xq", name="xq")
    xkv_h = singles.tile([128, 2, N], F16, tag="xkv_h", name="xkv_h")
    xq_h = singles.tile([128, 2, NQ], F16, tag="xq_h", name="xq_h")
    w1t = singles.tile([128, 2, CK], F16, tag="w1t", name="w1t")
    w2t = singles.tile([128, 2, CK], F16, tag="w2t", name="w2t")
    w3t = singles.tile([128, 2, C], F16, tag="w3t", name="w3t")
    g_sb = singles.tile([CK, N], F16, tag="g_sb", name="g_sb")
    f_sb = singles.tile([CK, NQ], F16, tag="f_sb", name="f_sb")
    V = singles.tile([128, MT, 260], BF16, tag="V", name="V")

    nc.vector.memset(V[:, :, 256:257], 1.0)

    # PSUM: one shared pool ("st" tag, 2-bank slots, bufs=3) hosts the S^T
    # tiles and all projection outputs; one 1-bank pool (bufs=2) for the PV
    # accumulators. 6 + 2 = 8 banks.
    stp = ctx.enter_context(tc.tile_pool(name="st_ps", bufs=3, space="PSUM"))
    op = ctx.enter_context(tc.tile_pool(name="o_ps", bufs=2, space="PSUM"))
    ptp = ctx.enter_context(tc.tile_pool(name="pt", bufs=2))
    osbp = ctx.enter_context(tc.tile_pool(name="osb", bufs=2))
    otp = ctx.enter_context(tc.tile_pool(name="ot", bufs=4))
    rp = ctx.enter_context(tc.tile_pool(name="r", bufs=2))
    stgp = ctx.enter_context(tc.tile_pool(name="stage", bufs=2))

    Pt = [None, None]
    stage = [None, None]
    posts = []

    def emit_post(item):
        cc, j, o_ps, stg = item
        J = cc * 4 + j
        r = rp.tile([128, 1], F32, tag="r", name="r")
        nc.vector.reciprocal(r[:], o_ps[:, 256:257])
        o_sb = osbp.tile([128, 256], BF16, tag="osb", name="osb")
        nc.vector.tensor_scalar_mul(o_sb[:], o_ps[:, 0:256], r[:])
        for h in range(2):
            ot = otp.tile([128, 128], BF16, tag="ot", name="ot")
            nc.sync.dma_start_transpose(ot[:], o_sb[:, ts(h, 128)])
            nc.vector.tensor_add(stg[:, h, ts(j, 128)], ot[:],
                                 xq[:, h, ds(J * 128, 128)])
        if j == 3:
            for k in range(2):
                nc.gpsimd.dma_start(out_d[ts(k, 128), ts(cc, 512)], stg[:, k, :])

    def st_group(c, gidx):
        st = stp.tile([128, 2, 512], F32, tag="st", name="st")
        for t in range(2):
            mt = 2 * gidx + t
            nc.tensor.matmul(st[:, t, :], g_sb[:, ts(mt, 128)],
                             f_sb[:, ts(c, 512)], start=True, stop=True)
        nc.scalar.activation(Pt[c % 2][:, 2 * gidx:2 * gidx + 2, :], st[:], EXP)

    # ---- load + cast (GpSimd) ----
    with tc.tile_pool(name="xraw", bufs=1) as xrp:
        xkv = xrp.tile([128, 2, N], F32, tag="xkv", name="xkv")
        wraw = xrp.tile([128, 2, 2 * CK + C], F32, tag="wraw", name="wraw")
        for k in range(2):
            nc.sync.dma_start(wraw[:, k, 0:CK], w1t_d[ts(k, 128), :])
            nc.sync.dma_start(wraw[:, k, CK:2 * CK], w2t_d[ts(k, 128), :])
            nc.sync.dma_start(wraw[:, k, 2 * CK:], w3t_d[ts(k, 128), :])
        nc.gpsimd.tensor_copy(w1t[:], wraw[:, :, 0:CK])
        nc.gpsimd.tensor_copy(w2t[:], wraw[:, :, CK:2 * CK])
        nc.gpsimd.tensor_copy(w3t[:], wraw[:, :, 2 * CK:])
        for ch in range(NQ // 512):
            for k in range(2):
                nc.sync.dma_start(xq[:, k, ts(ch, 512)], xq_d[ts(k, 128), ts(ch, 512)])
            nc.gpsimd.tensor_copy(xq_h[:, :, ts(ch, 512)], xq[:, :, ts(ch, 512)])
        for ch in range(N // 512):
            for k in range(2):
                nc.sync.dma_start(xkv[:, k, ts(ch, 512)],
                                  xkv_d[ts(k, 128), ts(ch, 512)])
            nc.gpsimd.tensor_copy(xkv_h[:, :, ts(ch, 512)], xkv[:, :, ts(ch, 512)])

        # ---- projections, interleaved with chunk 0 of the scores ----
        Pt[0] = ptp.tile([128, MT, 512], BF16, tag="pt", name="pt")
        for ch in range(NQ // 512):
            fp = stp.tile([CK, 512], F32, tag="st", name="fp")
            for k in range(2):
                nc.tensor.matmul(fp[:], w1t[:, k, :], xq_h[:, k, ts(ch, 512)],
                                 start=(k == 0), stop=(k == 1))
            nc.vector.tensor_copy(f_sb[:, ts(ch, 512)], fp[:])
        for ch in range(N // 512):
            gp = stp.tile([CK, 512], F32, tag="st", name="gp")
            for k in range(2):
                nc.tensor.matmul(gp[:], w2t[:, k, :], xkv_h[:, k, ts(ch, 512)],
                                 start=(k == 0), stop=(k == 1))
            nc.vector.tensor_copy(g_sb[:, ts(ch, 512)], gp[:])
            # scores for the m-tiles this g-chunk just produced
            st_group(0, 2 * ch)
            st_group(0, 2 * ch + 1)
        for mt in range(MT):
            vp = stp.tile([128, 256], F32, tag="st", name="vp")
            for k in range(2):
                nc.tensor.matmul(vp[:], xkv_h[:, k, ts(mt, 128)], w3t[:, k, :],
                                 start=(k == 0), stop=(k == 1))
            nc.vector.tensor_copy(V[:, mt, 0:256], vp[:])

    # ---- attention chunks 1..NCHUNK, software-pipelined by one chunk ----
    for c in range(1, NCHUNK + 1):
        if c < NCHUNK:
            Pt[c % 2] = ptp.tile([128, MT, 512], BF16, tag="pt", name="pt")
        stage[(c - 1) % 2] = stgp.tile([128, 2, 512], F32, tag="stage", name="stage")
        o_cur = None
        for gidx in range(16):
            if c < NCHUNK:
                st_group(c, gidx)
            j, seg = gidx // 4, gidx % 4
            if seg == 0:
                o_cur = op.tile([128, 257], F32, tag="o", name="o")
            for mm in range(8):
                mt = seg * 8 + mm
                nc.tensor.matmul(o_cur[:], Pt[(c - 1) % 2][:, mt, ts(j, 128)],
                                 V[:, mt, 0:257],
                                 start=(mt == 0), stop=(mt == MT - 1),
                                 skip_group_check=True)
            if seg == 3:
                posts.append((c - 1, j, o_cur, stage[(c - 1) % 2]))
            # delay each n-tile's post-processing by one PE group so the DVE
            # normalize never stalls the PE stream
            while len(posts) > 1:
                emit_post(posts.pop(0))
    while posts:
        emit_post(posts.pop(0))


_NC_CACHE = None


def _get_nc():
    global _NC_CACHE
    if _NC_CACHE is None:
        _NC_CACHE = build_nc()
    return _NC_CACHE


def make_in_maps(x, w1, w2, w3):
    x = np.ascontiguousarray(x, dtype=np.float32).reshape(B, C, N)
    w1t = np.ascontiguousarray(w1.T, dtype=np.float32)
    w2t = np.ascontiguousarray(w2.T, dtype=np.float32)
    w3t = np.ascontiguousarray(w3.T, dtype=np.float32)
    in_maps = []
    for core in range(8):
        b, half = core // 2, core % 2
        in_maps.append({
            "xkv": x[b],
            "xq": np.ascontiguousarray(x[b][:, half * NQ:(half + 1) * NQ]),
            "w1t": w1t,
            "w2t": w2t,
            "w3t": w3t,
        })
    return in_maps


def assemble(results):
    out = np.empty((B, C, N), dtype=np.float32)
    for core in range(8):
        b, half = core // 2, core % 2
        out[b][:, half * NQ:(half + 1) * NQ] = results[core]["out"]
    return out.reshape(B, C, H, W)


def kernel(x, w1, w2, w3):
    nc = _get_nc()
    res = run_bass_kernel_spmd(nc, make_in_maps(x, w1, w2, w3),
                               core_ids=list(range(8)))
    return assemble(res.results)


# revision 11
# speedup vs baseline: 1.5263x; 1.0845x over previous
"""Trainium2 Bass kernel for SAGAN-style spatial self-attention.

Reference computation (per batch b):
    xf = x[b].reshape(C, N)                    # C=256, N=64*64=4096
    f  = w1 @ xf                               # [32, N]   (query^T)
    g  = w2 @ xf                               # [32, N]   (key)
    V  = (w3 @ xf)^T                           # [N, C]    (value)
    S  = f^T @ g                               # [N, N]
    O  = softmax(S, axis=-1) @ V               # [N, C]
    out[b] = O^T.reshape(C, H, W) + x[b]

Sharding: 8 cores = 4 batches x 2 query-halves. Each core holds its batch's
full xf (for keys/values) and computes attention for 2048 query positions.
No cross-core communication.

Per-core device algorithm (n = this core's 2048 query cols, m = all 4096 keys):
  - projections f [32,2048], g [32,4096] in fp16; V [4096,257] in bf16
    (column 256 of V is ones -> PV matmul emits softmax denominator for free)
  - S^T chunks: matmul(lhsT=g_mtile [32,128], rhs=f_chunk [32,512]) -> PSUM
  - P^T = exp(S^T) -> SBUF bf16 (no max subtraction: |S| <~ 45, exp fits fp32)
  - O chunk: matmul(lhsT=P^T [128m,128n], rhs=V [128m,257]) accumulated over
    32 m-tiles -> [128n, 257]; r = 1/col256; O *= r (bf16)
  - DMA-transpose O to [C, n] layout, add residual xq, DMA out.

fp16 (not fp32/fp32r) operands everywhere on the PE: fp32-mode matmuls do not
register as PE-busy for the HAM clock gate and the PE gets stuck at 1.2GHz.
fp16 keeps full clock and has enough mantissa (2^-11) for the pre-exp scores.
"""

import sys

sys.path.insert(0, "/opt/trn_rl_repo")

from contextlib import ExitStack

import numpy as np

import concourse.bass as bass
import concourse.tile as tile
from concourse import bacc, mybir
from concourse.bass import ts, ds
from concourse.bass_utils import run_bass_kernel_spmd

F32 = mybir.dt.float32
F16 = mybir.dt.float16
BF16 = mybir.dt.bfloat16

B, C, H, W = 4, 256, 64, 64
N = H * W          # 4096 keys per batch
NQ = N // 2        # 2048 queries per core
CK = 32            # query/key head dim
MT = N // 128      # 32 m-tiles
NCHUNK = NQ // 512  # 4 n-chunks of 512 query cols
EXP = mybir.ActivationFunctionType.Exp


def build_nc():
    nc = bacc.Bacc("TRN2", target_bir_lowering=False, debug=False, num_devices=8)
    xkv_d = nc.dram_tensor("xkv", [C, N], F16, kind="ExternalInput")
    xq_d = nc.dram_tensor("xq", [C, NQ], F32, kind="ExternalInput")
    xqh_d = nc.dram_tensor("xqh", [C, NQ], F16, kind="ExternalInput")
    w1t_d = nc.dram_tensor("w1t", [C, CK], F16, kind="ExternalInput")
    w2t_d = nc.dram_tensor("w2t", [C, CK], F16, kind="ExternalInput")
    w3t_d = nc.dram_tensor("w3t", [C, C], F16, kind="ExternalInput")
    out_d = nc.dram_tensor("out", [C, NQ], F32, kind="ExternalOutput")

    with tile.TileContext(nc) as tc, ExitStack() as ctx:
        _body(ctx, tc, xkv_d.ap(), xq_d.ap(), xqh_d.ap(), w1t_d.ap(), w2t_d.ap(),
              w3t_d.ap(), out_d.ap())
    nc.compile()
    return nc


def _body(ctx, tc, xkv_d, xq_d, xqh_d, w1t_d, w2t_d, w3t_d, out_d):
    nc = tc.nc
    singles = ctx.enter_context(tc.tile_pool(name="singles", bufs=1))

    xq = singles.tile([128, 2, NQ], F32, tag="xq", name="xq")
    xkv_h = singles.tile([128, 2, N], F16, tag="xkv_h", name="xkv_h")
    xq_h = singles.tile([128, 2, NQ], F16, tag="xq_h", name="xq_h")
    w1t = singles.tile([128, 2, CK], F16, tag="w1t", name="w1t")
    w2t = singles.tile([128, 2, CK], F16, tag="w2t", name="w2t")
    w3t = singles.tile([128, 2, C], F16, tag="w3t", name="w3t")
    g_sb = singles.tile([CK, N], F16, tag="g_sb", name="g_sb")
    f_sb = singles.tile([CK, NQ], F16, tag="f_sb", name="f_sb")
    V = singles.tile([128, MT, 260], BF16, tag="V", name="V")

    nc.vector.memset(V[:, :, 256:257], 1.0)

    # PSUM: one shared pool ("st" tag, 2-bank slots, bufs=3) hosts the S^T
    # tiles and all projection outputs; one 1-bank pool (bufs=2) for the PV
    # accumulators. 6 + 2 = 8 banks.
    stp = ctx.enter_context(tc.tile_pool(name="st_ps", bufs=3, space="PSUM"))
    op = ctx.enter_context(tc.tile_pool(name="o_ps", bufs=2, space="PSUM"))
    ptp = ctx.enter_context(tc.tile_pool(name="pt", bufs=2))
    osbp = ctx.enter_context(tc.tile_pool(name="osb", bufs=2))
    otp = ctx.enter_context(tc.tile_pool(name="ot", bufs=4))
    rp = ctx.enter_context(tc.tile_pool(name="r", bufs=2))
    stgp = ctx.enter_context(tc.tile_pool(name="stage", bufs=2))

    Pt = [None, None]
    stage = [None, None]
    posts = []

    def emit_post(item):
        cc, j, o_ps, stg = item
        J = cc * 4 + j
        r = rp.tile([128, 1], F32, tag="r", name="r")
        nc.vector.reciprocal(r[:], o_ps[:, 256:257])
        o_sb = osbp.tile([128, 256], BF16, tag="osb", name="osb")
        nc.vector.tensor_scalar_mul(o_sb[:], o_ps[:, 0:256], r[:])
        for h in range(2):
            ot = otp.tile([128, 128], BF16, tag="ot", name="ot")
            nc.sync.dma_start_transpose(ot[:], o_sb[:, ts(h, 128)])
            nc.vector.tensor_add(stg[:, h, ts(j, 128)], ot[:],
                                 xq[:, h, ds(J * 128, 128)])
        if j == 3:
            for k in range(2):
                nc.gpsimd.dma_start(out_d[ts(k, 128), ts(cc, 512)], stg[:, k, :])

    def st_group(c, gidx):
        st = stp.tile([128, 2, 512], F32, tag="st", name="st")
        for t in range(2):
            mt = 2 * gidx + t
            nc.tensor.matmul(st[:, t, :], g_sb[:, ts(mt, 128)],
                             f_sb[:, ts(c, 512)], start=True, stop=True)
        nc.scalar.activation(Pt[c % 2][:, 2 * gidx:2 * gidx + 2, :], st[:], EXP)

    # ---- input DMAs (fp16 operands are cast host-side) ----
    for k in range(2):
        nc.sync.dma_start(w1t[:, k, :], w1t_d[ts(k, 128), :])
        nc.sync.dma_start(w2t[:, k, :], w2t_d[ts(k, 128), :])
        nc.sync.dma_start(w3t[:, k, :], w3t_d[ts(k, 128), :])
    for ch in range(NQ // 512):
        for k in range(2):
            nc.sync.dma_start(xq_h[:, k, ts(ch, 512)], xqh_d[ts(k, 128), ts(ch, 512)])
    for ch in range(N // 512):
        for k in range(2):
            nc.sync.dma_start(xkv_h[:, k, ts(ch, 512)],
                              xkv_d[ts(k, 128), ts(ch, 512)])
    for ch in range(NQ // 512):
        for k in range(2):
            nc.sync.dma_start(xq[:, k, ts(ch, 512)], xq_d[ts(k, 128), ts(ch, 512)])

    # ---- projections, interleaved with chunk 0 of the scores ----
    Pt[0] = ptp.tile([128, MT, 512], BF16, tag="pt", name="pt")
    for ch in range(NQ // 512):
        fp = stp.tile([CK, 512], F32, tag="st", name="fp")
        for k in range(2):
            nc.tensor.matmul(fp[:], w1t[:, k, :], xq_h[:, k, ts(ch, 512)],
                             start=(k == 0), stop=(k == 1))
        nc.vector.tensor_copy(f_sb[:, ts(ch, 512)], fp[:])
    for ch in range(N // 512):
        gp = stp.tile([CK, 512], F32, tag="st", name="gp")
        for k in range(2):
            nc.tensor.matmul(gp[:], w2t[:, k, :], xkv_h[:, k, ts(ch, 512)],
                             start=(k == 0), stop=(k == 1))
        nc.vector.tensor_copy(g_sb[:, ts(ch, 512)], gp[:])
        # scores for the m-tiles this g-chunk just produced
        st_group(0, 2 * ch)
        st_group(0, 2 * ch + 1)
    for mt in range(MT):
        vp = stp.tile([128, 256], F32, tag="st", name="vp")
        for k in range(2):
            nc.tensor.matmul(vp[:], xkv_h[:, k, ts(mt, 128)], w3t[:, k, :],
                             start=(k == 0), stop=(k == 1))
        nc.vector.tensor_copy(V[:, mt, 0:256], vp[:])

    # ---- attention chunks 1..NCHUNK, software-pipelined by one chunk ----
    for c in range(1, NCHUNK + 1):
        if c < NCHUNK:
            Pt[c % 2] = ptp.tile([128, MT, 512], BF16, tag="pt", name="pt")
        stage[(c - 1) % 2] = stgp.tile([128, 2, 512], F32, tag="stage", name="stage")
        o_cur = None
        for gidx in range(16):
            if c < NCHUNK:
                st_group(c, gidx)
            j, seg = gidx // 4, gidx % 4
            if seg == 0:
                o_cur = op.tile([128, 257], F32, tag="o", name="o")
            for mm in range(8):
                mt = seg * 8 + mm
                nc.tensor.matmul(o_cur[:], Pt[(c - 1) % 2][:, mt, ts(j, 128)],
                                 V[:, mt, 0:257],
                                 start=(mt == 0), stop=(mt == MT - 1),
                                 skip_group_check=True)
            if seg == 3:
                posts.append((c - 1, j, o_cur, stage[(c - 1) % 2]))
            # delay each n-tile's post-processing by one PE group so the DVE
            # normalize never stalls the PE stream
            while len(posts) > 1:
                emit_post(posts.pop(0))
    while posts:
        emit_post(posts.pop(0))


_NC_CACHE = None


def _get_nc():
    global _NC_CACHE
    if _NC_CACHE is None:
        _NC_CACHE = build_nc()
    return _NC_CACHE


def make_in_maps(x, w1, w2, w3):
    x = np.ascontiguousarray(x, dtype=np.float32).reshape(B, C, N)
    w1t = np.ascontiguousarray(w1.T, dtype=np.float32)
    w2t = np.ascontiguousarray(w2.T, dtype=np.float32)
    w3t = np.ascontiguousarray(w3.T, dtype=np.float32)
    in_maps = []
    xh = x.astype(np.float16)
    for core in range(8):
        b, half = core // 2, core % 2
        xq_core = np.ascontiguousarray(x[b][:, half * NQ:(half + 1) * NQ])
        in_maps.append({
            "xkv": xh[b],
            "xq": xq_core,
            "xqh": np.ascontiguousarray(xh[b][:, half * NQ:(half + 1) * NQ]),
            "w1t": w1t.astype(np.float16),
            "w2t": w2t.astype(np.float16),
            "w3t": w3t.astype(np.float16),
        })
    return in_maps


def assemble(results):
    out = np.empty((B, C, N), dtype=np.float32)
    for core in range(8):
        b, half = core // 2, core % 2
        out[b][:, half * NQ:(half + 1) * NQ] = results[core]["out"]
    return out.reshape(B, C, H, W)


def kernel(x, w1, w2, w3):
    nc = _get_nc()
    res = run_bass_kernel_spmd(nc, make_in_maps(x, w1, w2, w3),
                               core_ids=list(range(8)))
    return assemble(res.results)
